# revision 1
# baseline (speedup 1.0000x reference)
"""Trainium2 Bass kernel for JointSelfAttention (B=4,T=2048,C=1024,H=16).

Sharding: 8 cores = 4 batches (data-parallel) x 2 head-groups of 8 heads
(tensor-parallel).  Each core computes qkv for its head group, qk-RMSNorm,
RoPE, causal attention, and a partial c_proj; the host sums the two partial
projections per batch (the contraction over head-group channels) and
transposes back.

Per-core layouts are transposed (channels x T) so every matmul's contraction
dim lands on SBUF partitions with no on-device transposes except the q/k
head transposes (PE-mode, needed to get q/k into (hd x T) for scores).
Softmax is max-free: q/k are RMS-normalised so scores are bounded by
sqrt(hd)=8 in magnitude, making exp safe in fp32.  Scores are computed
transposed (k on partitions), so the softmax denominator is folded into the
attn@v matmul via a ones-column appended to v.
"""

import math
import numpy as np
from contextlib import ExitStack

B, T, C, H, HD = 4, 2048, 1024, 16, 64
HG = 2              # head groups (tensor-parallel dim)
HPG = H // HG       # heads per group = 8
CG = HPG * HD       # channels per group = 512
N_CORES = B * HG
EPS = float(np.finfo(np.float32).eps)
QW = 512            # query window (free dim per attention block)
NQW = T // QW       # 4 windows
NKT = T // 128      # 16 k tiles
NMT = T // 128      # 16 m (token) tiles
NKC = C // 128      # 8 contraction tiles for qkv


def _split_excess_waits(nc, mybir, max_waits=1):
    """This container's walrus only encodes 1 sync-wait per instruction
    ("Too many sync wait commands" in CoreV3 codegen).  Move extra waits to
    preceding NoOps on the same engine."""
    for f in nc.m.functions:
        for bb in f.blocks:
            new_insts = []
            for inst in bb.instructions:
                si = inst.sync_info
                if si is not None and si.on_wait and len(si.on_wait) > max_waits:
                    waits = list(si.on_wait)
                    extra, keep = waits[:-max_waits], waits[-max_waits:]
                    for i in range(0, len(extra), max_waits):
                        nop = mybir.InstNoOp(
                            name=f"{inst.name}-ws{i}", ins=[], outs=[])
                        nop.engine = inst.engine
                        nop.sync_info = mybir.SyncInfo(
                            on_wait=extra[i:i + max_waits], on_update=[])
                        new_insts.append(nop)
                    inst.sync_info = mybir.SyncInfo(
                        on_wait=keep, on_update=list(si.on_update or []))
                new_insts.append(inst)
            bb.instructions.clear()
            bb.instructions.extend(new_insts)


def _build_nc():
    import concourse.bass as bass
    import concourse.tile as tile
    from concourse import mybir
    from concourse.masks import make_identity

    f32 = mybir.dt.float32
    f32r = mybir.dt.float32r
    AF = mybir.ActivationFunctionType

    def r(ap):  # matmul operands are natively f32r (walrus requires
        return ap  # producers to round-to-f32r; tiles are declared f32r)

    nc = bass.Bass("TRN2", debug=False, num_devices=N_CORES)

    xt = nc.dram_tensor("xt", [NKC, NMT, 128, 128], f32r, kind="ExternalInput").ap()
    wqk = nc.dram_tensor("wqk", [C, 2 * CG], f32r, kind="ExternalInput").ap()
    wv = nc.dram_tensor("wv", [C, CG], f32r, kind="ExternalInput").ap()
    wp = nc.dram_tensor("wp", [CG, C], f32r, kind="ExternalInput").ap()
    cosn = nc.dram_tensor("cosn", [128, NMT, HD // 2], f32, kind="ExternalInput").ap()
    sinn = nc.dram_tensor("sinn", [128, NMT, HD // 2], f32, kind="ExternalInput").ap()
    trim = nc.dram_tensor("trim", [128, 128], f32r, kind="ExternalInput").ap()
    onesd = nc.dram_tensor("onesd", [128, 128], f32r, kind="ExternalInput").ap()
    out = nc.dram_tensor("o", [C, T], f32, kind="ExternalOutput").ap()

    with tile.TileContext(nc) as tc:
        with ExitStack() as ctx:
            # ---- persistent buffers (live across all phases) ----
            persist = ctx.enter_context(tc.tile_pool(name="persist", bufs=1))
            qT = persist.tile([128, HPG // 2, T], f32r)   # q heads (hd x T); head h at [(h%2)*64:, h//2, :]
            kT = persist.tile([128, HPG // 2, T], f32r)
            vaug = persist.tile([128, NKT, HPG, HD + 1], f32r)  # v + ones col per (ktile, head)
            cos_sb = persist.tile([128, NMT, HD // 2], f32)
            sin_sb = persist.tile([128, NMT, HD // 2], f32)
            trim_sb = persist.tile([128, 128], f32r)
            ident = persist.tile([128, 128], f32)
            eps_sb = persist.tile([128, 1], f32)
            ones_sb = persist.tile([1, 64], f32r)

            nc.sync.dma_start(cos_sb[:], cosn[:])
            nc.sync.dma_start(sin_sb[:], sinn[:])
            nc.sync.dma_start(trim_sb[:], trim[:])
            make_identity(nc, ident[:])
            nc.vector.memset(eps_sb[:], EPS)
            # f32r memset is not encodable; DMA ones from DRAM instead
            nc.sync.dma_start(ones_sb[:], onesd[0:1, 0:64])
            nc.sync.dma_start(
                vaug[:, :, :, HD:HD + 1],
                onesd[:, 0:NKT * HPG].rearrange("p (a b) -> p a b", b=HPG))

            # ---- phase 1: qkv + norm + rope + transpose ----
            with ExitStack() as p1:
                wpool = p1.enter_context(tc.tile_pool(name="w1", bufs=1))
                wqk_sb = wpool.tile([128, NKC, 2 * CG], f32r)
                wv_sb = wpool.tile([128, NKC, CG], f32r)
                nc.sync.dma_start(wqk_sb[:], wqk.rearrange("(kc p) n -> p kc n", p=128))
                nc.sync.dma_start(wv_sb[:], wv.rearrange("(kc p) n -> p kc n", p=128))

                xpool = p1.enter_context(tc.tile_pool(name="xp", bufs=3))
                qkps = p1.enter_context(tc.tile_pool(name="qkps", bufs=2, space="PSUM"))
                vps = p1.enter_context(tc.tile_pool(name="vps", bufs=2, space="PSUM"))
                tpps = p1.enter_context(tc.tile_pool(name="tpps", bufs=2, space="PSUM"))
                work = p1.enter_context(tc.tile_pool(name="work", bufs=2))
                stats = p1.enter_context(tc.tile_pool(name="stats", bufs=2))

                for mt in range(NMT):
                    xt_sb = xpool.tile([128, NKC, 128], f32r)
                    for kc in range(NKC):
                        nc.sync.dma_start(xt_sb[:, kc, :], xt[kc, mt])

                    qk_ps = qkps.tile([128, 2 * CG], f32)
                    for n in range(2):
                        for kc in range(NKC):
                            nc.tensor.matmul(
                                qk_ps[:, n * CG:(n + 1) * CG],
                                r(xt_sb[:, kc, :]),
                                r(wqk_sb[:, kc, n * CG:(n + 1) * CG]),
                                start=(kc == 0), stop=(kc == NKC - 1))
                    v_ps = vps.tile([128, CG], f32)
                    for kc in range(NKC):
                        nc.tensor.matmul(
                            v_ps[:], r(xt_sb[:, kc, :]), r(wv_sb[:, kc, :]),
                            start=(kc == 0), stop=(kc == NKC - 1))

                    # v -> vaug (strided per-head copy, leaves ones col intact)
                    nc.vector.tensor_copy(
                        vaug[:, mt, :, 0:HD],
                        v_ps[:].rearrange("p (h d) -> p h d", d=HD))

                    # qk RMS norm: rr = 1/sqrt(mean(x^2)+eps) per (token, head)
                    sq = work.tile([128, 2 * CG], f32)
                    nc.scalar.activation(sq[:], qk_ps[:], AF.Square)
                    ss = stats.tile([128, 2 * HPG], f32)
                    nc.vector.tensor_reduce(
                        ss[:], sq[:].rearrange("p (h d) -> p h d", d=HD),
                        axis=mybir.AxisListType.X, op=mybir.AluOpType.add)
                    rr = stats.tile([128, 2 * HPG], f32)
                    nc.scalar.activation(rr[:], ss[:], AF.Sqrt,
                                         bias=eps_sb[:], scale=1.0 / HD)
                    nc.vector.reciprocal(rr[:], rr[:])
                    qkn = work.tile([128, 2 * CG], f32)
                    rr_b = bass.AP(tensor=rr.tensor, offset=rr.offset,
                                   ap=[rr.ap[0], [1, 2 * HPG], [0, HD]])
                    nc.vector.tensor_tensor(
                        qkn[:].rearrange("p (h d) -> p h d", d=HD),
                        qk_ps[:].rearrange("p (h d) -> p h d", d=HD),
                        rr_b, op=mybir.AluOpType.mult)

                    # rope on q and k halves: y1 = x1 c + x2 s ; y2 = x2 c - x1 s
                    # (written back into qkn: the adds/subs only read the tmps)
                    qkr = qkn
                    cb = bass.AP(tensor=cos_sb.tensor,
                                 offset=cos_sb.offset + mt * (HD // 2),
                                 ap=[cos_sb.ap[0], [0, HPG], [1, HD // 2]])
                    sb_ = bass.AP(tensor=sin_sb.tensor,
                                  offset=sin_sb.offset + mt * (HD // 2),
                                  ap=[sin_sb.ap[0], [0, HPG], [1, HD // 2]])
                    for sec in range(2):  # 0 = q, 1 = k
                        base = qkn[:, sec * CG:(sec + 1) * CG].rearrange(
                            "p (h two d) -> p h two d", two=2, d=HD // 2)
                        dst = qkr[:, sec * CG:(sec + 1) * CG].rearrange(
                            "p (h two d) -> p h two d", two=2, d=HD // 2)
                        x1, x2 = base[:, :, 0, :], base[:, :, 1, :]
                        t1 = work.tile([128, HPG, HD // 2], f32, tag="rt1")
                        t2 = work.tile([128, HPG, HD // 2], f32, tag="rt2")
                        t3 = work.tile([128, HPG, HD // 2], f32, tag="rt3")
                        t4 = work.tile([128, HPG, HD // 2], f32, tag="rt4")
                        nc.vector.tensor_tensor(t1[:], x1, cb, op=mybir.AluOpType.mult)
                        nc.vector.tensor_tensor(t2[:], x2, sb_, op=mybir.AluOpType.mult)
                        nc.vector.tensor_tensor(t3[:], x2, cb, op=mybir.AluOpType.mult)
                        nc.vector.tensor_tensor(t4[:], x1, sb_, op=mybir.AluOpType.mult)
                        nc.vector.tensor_add(dst[:, :, 0, :], t1[:], t2[:])
                        nc.vector.tensor_sub(dst[:, :, 1, :], t3[:], t4[:])

                    # transpose each head (128 t x 64 d) -> (64 d x 128 t)
                    for sec, dstT in ((0, qT), (1, kT)):
                        for hh in range(0, HPG, 4):
                            tp = tpps.tile([64, 4, 128], f32, tag="tp")
                            for j in range(4):
                                h = hh + j
                                nc.tensor.transpose(
                                    tp[:, j, :],
                                    qkr[:, sec * CG + h * HD: sec * CG + (h + 1) * HD],
                                    ident[:])
                            for j in range(4):
                                h = hh + j
                                eng = nc.vector if (h % 2 == 0) else nc.scalar
                                dst = dstT[(h % 2) * 64:(h % 2) * 64 + 64,
                                           h // 2, mt * 128:(mt + 1) * 128]
                                if h % 2 == 0:
                                    nc.vector.tensor_copy(dst, tp[:, j, :])
                                else:
                                    nc.scalar.copy(dst, tp[:, j, :])

            # ---- phases 2+3 share ybuf (allocated after phase-1 weights free) ----
            ybufp = ctx.enter_context(tc.tile_pool(name="ybufp", bufs=1))
            ybuf = ybufp.tile([128, CG // 128, T], f32r)  # attn out (ch x T), proj rhs

            # ---- phase 2: attention per (head, q-window) ----
            with ExitStack() as p2:
                stps = p2.enter_context(tc.tile_pool(name="stps", bufs=3, space="PSUM"))
                ytps = p2.enter_context(tc.tile_pool(name="ytps", bufs=2, space="PSUM"))
                bcps = p2.enter_context(tc.tile_pool(name="bcps", bufs=2, space="PSUM"))
                ptpool = p2.enter_context(tc.tile_pool(name="pt", bufs=3))
                epil = p2.enter_context(tc.tile_pool(name="epil", bufs=2))

                for h in range(HPG):
                    po = (h % 2) * 64
                    tr = h // 2
                    for qw in range(NQW):
                        n_kt = 4 * qw + 4
                        y_ps = ytps.tile([65, QW], f32)
                        for kt in range(n_kt):
                            d = kt - 4 * qw
                            col0 = d * 128 if d >= 0 else 0
                            ncols = QW - col0
                            st = stps.tile([128, QW], f32, tag="st")
                            nc.tensor.matmul(
                                st[:, col0:],
                                r(kT[po:po + 64, tr, kt * 128:(kt + 1) * 128]),
                                r(qT[po:po + 64, tr, qw * QW + col0: (qw + 1) * QW]),
                                start=True, stop=True)
                            pt = ptpool.tile([128, QW], f32r, tag="pt")
                            nc.scalar.activation(pt[:, col0:], st[:, col0:],
                                                 AF.Exp, scale=1.0 / math.sqrt(HD))
                            if d >= 0:
                                nc.gpsimd.tensor_mul(
                                    pt[:, col0:col0 + 128],
                                    pt[:, col0:col0 + 128], trim_sb[:])
                            nc.tensor.matmul(
                                y_ps[:, col0:],
                                r(vaug[:, kt, h, :]),
                                r(pt[:, col0:]),
                                start=(kt == 0), stop=(kt == n_kt - 1))

                        # epilogue: divide by denominator (row 64 of y_ps);
                        # broadcast 1/denom across partitions via K=1 matmul
                        rd = epil.tile([1, QW], f32r, tag="rd")
                        with nc.allow_low_precision(reason="f32r matmul input"):
                            nc.vector.reciprocal(rd[:], y_ps[64:65, :])
                        bc_ps = bcps.tile([64, QW], f32)
                        nc.tensor.matmul(bc_ps[:], r(ones_sb[:]), r(rd[:]),
                                         start=True, stop=True)
                        bc = epil.tile([64, QW], f32, tag="bc")
                        nc.scalar.copy(bc[:], bc_ps[:])
                        nc.vector.tensor_tensor(
                            ybuf[po:po + 64, tr, qw * QW:(qw + 1) * QW],
                            y_ps[0:64, :], bc[:], op=mybir.AluOpType.mult)

            # ---- phase 3: partial c_proj (out^T = wp^T @ ybuf) ----
            with ExitStack() as p3:
                wppool = p3.enter_context(tc.tile_pool(name="wp", bufs=1))
                wp_sb = wppool.tile([128, CG // 128, C], f32r)
                nc.sync.dma_start(wp_sb[:], wp.rearrange("(kc p) n -> p kc n", p=128))
                ops = p3.enter_context(tc.tile_pool(name="ops", bufs=3, space="PSUM"))
                ostg = p3.enter_context(tc.tile_pool(name="ostg", bufs=3))
                for mo in range(C // 128):
                    for qw in range(NQW):
                        po_ps = ops.tile([128, QW], f32)
                        for kc in range(CG // 128):
                            nc.tensor.matmul(
                                po_ps[:],
                                r(wp_sb[:, kc, mo * 128:(mo + 1) * 128]),
                                r(ybuf[:, kc, qw * QW:(qw + 1) * QW]),
                                start=(kc == 0), stop=(kc == CG // 128 - 1))
                        ot = ostg.tile([128, QW], f32)
                        if (mo + qw) % 2 == 0:
                            nc.scalar.copy(ot[:], po_ps[:])
                        else:
                            nc.vector.tensor_copy(ot[:], po_ps[:])
                        nc.sync.dma_start(
                            out[mo * 128:(mo + 1) * 128, qw * QW:(qw + 1) * QW],
                            ot[:])

    _split_excess_waits(nc, mybir)
    return nc


_NC_CACHE = {}


def _get_nc():
    if "nc" not in _NC_CACHE:
        _NC_CACHE["nc"] = _build_nc()
    return _NC_CACHE["nc"]


def _host_inputs(x, w_attn, w_proj):
    import ml_dtypes
    inv_freq = 1.0 / (10000.0 ** (np.arange(0, HD, 2, dtype=np.float32) / HD))
    t = np.arange(T, dtype=np.float32)
    freqs = np.outer(t, inv_freq)
    cos = np.cos(freqs).astype(ml_dtypes.bfloat16).astype(np.float32)
    sin = np.sin(freqs).astype(ml_dtypes.bfloat16).astype(np.float32)
    cosn = np.ascontiguousarray(cos.reshape(NMT, 128, HD // 2).transpose(1, 0, 2))
    sinn = np.ascontiguousarray(sin.reshape(NMT, 128, HD // 2).transpose(1, 0, 2))
    trim = np.triu(np.ones((128, 128), dtype=np.float32))
    ones_hd = np.ones((128, 128), dtype=np.float32)

    in_maps = []
    for b in range(B):
        xT = np.ascontiguousarray(x[b].T)  # (C, T)
        xt = np.ascontiguousarray(
            xT.reshape(NKC, 128, NMT, 128).transpose(0, 2, 1, 3))
        for hg in range(HG):
            qr = slice(hg * CG, (hg + 1) * CG)
            kr = slice(C + hg * CG, C + (hg + 1) * CG)
            vr = slice(2 * C + hg * CG, 2 * C + (hg + 1) * CG)
            wqk = np.ascontiguousarray(
                np.concatenate([w_attn[qr], w_attn[kr]], axis=0).T)
            wv = np.ascontiguousarray(w_attn[vr].T)
            wp = np.ascontiguousarray(w_proj[:, hg * CG:(hg + 1) * CG].T)
            in_maps.append({
                "xt": xt, "wqk": wqk, "wv": wv, "wp": wp,
                "cosn": cosn, "sinn": sinn, "trim": trim, "onesd": ones_hd,
            })
    return in_maps


def kernel(x, w_attn, w_proj, _profile=False):
    from concourse.bass_utils import run_bass_kernel_spmd
    nc = _get_nc()
    in_maps = _host_inputs(
        np.asarray(x, dtype=np.float32),
        np.asarray(w_attn, dtype=np.float32),
        np.asarray(w_proj, dtype=np.float32))
    res = run_bass_kernel_spmd(nc, in_maps, core_ids=list(range(N_CORES)),
                               trace=_profile)
    out = np.empty((B, T, C), dtype=np.float32)
    for b in range(B):
        acc = res.results[2 * b]["o"] + res.results[2 * b + 1]["o"]
        out[b] = acc.T
    if _profile:
        return out, res
    return out



# revision 7
# speedup vs baseline: 1.2576x; 1.2576x over previous
"""Trainium2 Bass kernel for JointSelfAttention (B=4,T=2048,C=1024,H=16).

Sharding: 8 cores = 4 batches (data-parallel) x 2 head-groups of 8 heads
(tensor-parallel).  Each core computes qkv for its head group, qk-RMSNorm,
RoPE, causal attention, and a partial c_proj; the host sums the two partial
projections per batch and transposes back.

v2: bf16 compute (matmul 1.0 cyc/row, DVE 2x modes, half DMA), paired head
transposes ([128,(2h,64d)] -> [128,128], halving PE transpose rows), exp
batched x4 over PSUM score groups, Act engine stripped to exp+sqrt only,
elementwise work spread across DVE/Pool.  Softmax stays max-free (scores
bounded by sqrt(hd)=8), denominator folded into attn@v via a ones column.
"""

import math
import numpy as np
from contextlib import ExitStack

B, T, C, H, HD = 4, 2048, 1024, 16, 64
HG = 2              # head groups (tensor-parallel dim)
HPG = H // HG       # heads per group = 8
CG = HPG * HD       # channels per group = 512
N_CORES = B * HG
EPS = float(np.finfo(np.float32).eps)
QW = 512            # query window (free dim per attention block)
NQW = T // QW       # 4 windows
NKT = T // 128      # 16 k tiles
NMT = T // 128      # 16 m (token) tiles
NKC = C // 128      # 8 contraction tiles for qkv


def _split_excess_waits(nc, mybir, max_waits=1):
    """This container's walrus only encodes 1 sync-wait per instruction
    ("Too many sync wait commands" in CoreV3 codegen).  Move extra waits to
    preceding NoOps on the same engine."""
    for f in nc.m.functions:
        for bb in f.blocks:
            new_insts = []
            for inst in bb.instructions:
                si = inst.sync_info
                if si is not None and si.on_wait and len(si.on_wait) > max_waits:
                    waits = list(si.on_wait)
                    extra, keep = waits[:-max_waits], waits[-max_waits:]
                    for i in range(0, len(extra), max_waits):
                        nop = mybir.InstNoOp(
                            name=f"{inst.name}-ws{i}", ins=[], outs=[])
                        nop.engine = inst.engine
                        nop.sync_info = mybir.SyncInfo(
                            on_wait=extra[i:i + max_waits], on_update=[])
                        new_insts.append(nop)
                    inst.sync_info = mybir.SyncInfo(
                        on_wait=keep, on_update=list(si.on_update or []))
                new_insts.append(inst)
            bb.instructions.clear()
            bb.instructions.extend(new_insts)


def _build_nc():
    import concourse.bass as bass
    import concourse.tile as tile
    from concourse import mybir
    from concourse.masks import make_identity

    f32 = mybir.dt.float32
    bf16 = mybir.dt.bfloat16
    AF = mybir.ActivationFunctionType
    MUL = mybir.AluOpType.mult

    nc = bass.Bass("TRN2", debug=False, num_devices=N_CORES)

    xt = nc.dram_tensor("xt", [NKC, NMT, 128, 128], bf16, kind="ExternalInput").ap()
    wqk = nc.dram_tensor("wqk", [C, 2 * CG], bf16, kind="ExternalInput").ap()
    wv = nc.dram_tensor("wv", [C, CG], bf16, kind="ExternalInput").ap()
    wp = nc.dram_tensor("wp", [CG, C], bf16, kind="ExternalInput").ap()
    cosn = nc.dram_tensor("cosn", [128, NMT, HD // 2], bf16, kind="ExternalInput").ap()
    sinn = nc.dram_tensor("sinn", [128, NMT, HD // 2], bf16, kind="ExternalInput").ap()
    trim = nc.dram_tensor("trim", [128, 128], bf16, kind="ExternalInput").ap()
    out = nc.dram_tensor("o", [C, T], f32, kind="ExternalOutput").ap()

    with tile.TileContext(nc) as tc:
        with ExitStack() as ctx:
            # ---- persistent buffers (live across all phases) ----
            persist = ctx.enter_context(tc.tile_pool(name="persist", bufs=1))
            # q/k heads transposed: head h lives at partitions (h%2)*64..+64,
            # free slot h//2 -> [128, 4, T]
            qT = persist.tile([128, HPG // 2, T], bf16)
            kT = persist.tile([128, HPG // 2, T], bf16)
            vaug = persist.tile([128, NKT, HPG, HD + 1], bf16)  # v + ones col
            cos_sb = persist.tile([128, NMT, HD // 2], bf16)
            sin_sb = persist.tile([128, NMT, HD // 2], bf16)
            trim_sb = persist.tile([128, 128], bf16)
            ident = persist.tile([128, 128], bf16)
            eps_sb = persist.tile([128, 1], f32)
            ones_sb = persist.tile([1, 64], bf16)

            nc.sync.dma_start(cos_sb[:], cosn[:])
            nc.sync.dma_start(sin_sb[:], sinn[:])
            nc.sync.dma_start(trim_sb[:], trim[:])
            make_identity(nc, ident[:])
            nc.vector.memset(eps_sb[:], EPS)
            nc.vector.memset(ones_sb[:], 1.0)
            nc.gpsimd.memset(
                vaug[:, :, :, HD:HD + 1].rearrange("p a b one -> p (a b one)"), 1.0)

            # ---- phase 1: qkv + norm + rope + transpose ----
            with ExitStack() as p1:
                wpool = p1.enter_context(tc.tile_pool(name="w1", bufs=1))
                wqk_sb = wpool.tile([128, NKC, 2 * CG], bf16)
                wv_sb = wpool.tile([128, NKC, CG], bf16)
                nc.sync.dma_start(wqk_sb[:], wqk.rearrange("(kc p) n -> p kc n", p=128))
                nc.sync.dma_start(wv_sb[:], wv.rearrange("(kc p) n -> p kc n", p=128))

                xpool = p1.enter_context(tc.tile_pool(name="xp", bufs=3))
                qkps = p1.enter_context(tc.tile_pool(name="qkps", bufs=2, space="PSUM"))
                vps = p1.enter_context(tc.tile_pool(name="vps", bufs=2, space="PSUM"))
                tpps = p1.enter_context(tc.tile_pool(name="tpps", bufs=2, space="PSUM"))
                work = p1.enter_context(tc.tile_pool(name="work", bufs=2))
                stats = p1.enter_context(tc.tile_pool(name="stats", bufs=2))

                for mt in range(NMT):
                    xt_sb = xpool.tile([128, NKC, 128], bf16)
                    for kc in range(NKC):
                        nc.sync.dma_start(xt_sb[:, kc, :], xt[kc, mt])

                    qk_ps = qkps.tile([128, 2 * CG], f32)
                    for n in range(2):
                        for kc in range(NKC):
                            nc.tensor.matmul(
                                qk_ps[:, n * CG:(n + 1) * CG],
                                xt_sb[:, kc, :],
                                wqk_sb[:, kc, n * CG:(n + 1) * CG],
                                start=(kc == 0), stop=(kc == NKC - 1))
                    v_ps = vps.tile([128, CG], f32)
                    for kc in range(NKC):
                        nc.tensor.matmul(
                            v_ps[:], xt_sb[:, kc, :], wv_sb[:, kc, :],
                            start=(kc == 0), stop=(kc == NKC - 1))

                    # v -> vaug (strided per-head copy, leaves ones col intact)
                    nc.scalar.copy(
                        vaug[:, mt, :, 0:HD],
                        v_ps[:].rearrange("p (h d) -> p h d", d=HD))

                    # qk RMS norm: rr = 1/sqrt(mean(x^2)+eps) per (token, head)
                    sq = work.tile([128, 2 * CG], bf16, tag="sq")
                    nc.scalar.activation(sq[:], qk_ps[:], AF.Square)
                    ss = stats.tile([128, 2 * HPG], f32)
                    nc.vector.tensor_reduce(
                        ss[:], sq[:].rearrange("p (h d) -> p h d", d=HD),
                        axis=mybir.AxisListType.X, op=mybir.AluOpType.add)
                    rr = stats.tile([128, 2 * HPG], f32)
                    nc.scalar.activation(rr[:], ss[:], AF.Sqrt,
                                         bias=eps_sb[:], scale=1.0 / HD)
                    nc.vector.reciprocal(rr[:], rr[:])
                    # norm multiply doubles as PSUM->SBUF bf16 conversion
                    qkn = work.tile([128, 2 * CG], bf16, tag="qkn")
                    rr_b = bass.AP(tensor=rr.tensor, offset=rr.offset,
                                   ap=[rr.ap[0], [1, 2 * HPG], [0, HD]])
                    nc.vector.tensor_tensor(
                        qkn[:].rearrange("p (h d) -> p h d", d=HD),
                        qk_ps[:].rearrange("p (h d) -> p h d", d=HD),
                        rr_b, op=MUL)

                    # rope on q and k at once: view [128, (2sec 8h 2half 32)]
                    # y1 = x1 c + x2 s ; y2 = x2 c - x1 s  (write back in place;
                    # the adds/subs only read the tmps)
                    base = qkn[:].rearrange(
                        "p (s h two d) -> p s h two d", s=2, h=HPG, two=2)
                    x1, x2 = base[:, :, :, 0, :], base[:, :, :, 1, :]
                    cb = bass.AP(tensor=cos_sb.tensor,
                                 offset=cos_sb.offset + mt * (HD // 2),
                                 ap=[cos_sb.ap[0], [0, 2], [0, HPG], [1, HD // 2]])
                    sb_ = bass.AP(tensor=sin_sb.tensor,
                                  offset=sin_sb.offset + mt * (HD // 2),
                                  ap=[sin_sb.ap[0], [0, 2], [0, HPG], [1, HD // 2]])
                    t1 = work.tile([128, 2, HPG, HD // 2], bf16, tag="rt1")
                    t2 = work.tile([128, 2, HPG, HD // 2], bf16, tag="rt2")
                    t3 = work.tile([128, 2, HPG, HD // 2], bf16, tag="rt3")
                    t4 = work.tile([128, 2, HPG, HD // 2], bf16, tag="rt4")
                    nc.vector.tensor_tensor(t1[:], x1, cb, op=MUL)
                    nc.vector.tensor_tensor(t2[:], x2, sb_, op=MUL)
                    nc.vector.tensor_tensor(t3[:], x2, cb, op=MUL)
                    nc.vector.tensor_tensor(t4[:], x1, sb_, op=MUL)
                    nc.vector.tensor_add(x1, t1[:], t2[:])
                    nc.vector.tensor_sub(x2, t3[:], t4[:])

                    # transpose head pairs: [128 tok, (2h, 64d)] -> [128, 128]
                    # (partitions 0-63 = head 2j dims, 64-127 = head 2j+1)
                    for sec, dstT in ((0, qT), (1, kT)):
                        tp = tpps.tile([128, HPG // 2, 128], bf16, tag="tp")
                        for j in range(HPG // 2):
                            src = qkn[:, sec * CG + 2 * j * HD:
                                      sec * CG + (2 * j + 2) * HD]
                            nc.tensor.transpose(
                                tp[:, j, :],
                                src.rearrange("p (two d) -> p two d", two=2),
                                ident[:])
                        nc.vector.tensor_copy(
                            dstT[:, :, mt * 128:(mt + 1) * 128], tp[:])

            # ---- phases 2+3 share ybuf (allocated after phase-1 weights free) ----
            ybufp = ctx.enter_context(tc.tile_pool(name="ybufp", bufs=1))
            ybuf = ybufp.tile([128, CG // 128, T], bf16)  # attn out (ch x T)

            # ---- phase 2: attention per (head, q-window) ----
            with ExitStack() as p2:
                stps = p2.enter_context(tc.tile_pool(name="stps", bufs=2, space="PSUM"))
                ytps = p2.enter_context(tc.tile_pool(name="ytps", bufs=2, space="PSUM"))
                bcps = p2.enter_context(tc.tile_pool(name="bcps", bufs=2, space="PSUM"))
                ptpool = p2.enter_context(tc.tile_pool(name="pt", bufs=3))
                epil = p2.enter_context(tc.tile_pool(name="epil", bufs=2))

                for h in range(HPG):
                    po = (h % 2) * 64
                    tr = h // 2
                    for qw in range(NQW):
                        n_kt = 4 * qw + 4
                        y_ps = ytps.tile([65, QW], f32)
                        for g in range(n_kt // 2):
                            st = stps.tile([128, 2, QW], f32, tag="st")
                            pt = ptpool.tile([128, 2, QW], bf16, tag="pt")
                            for j in range(2):
                                kt = 2 * g + j
                                d = kt - 4 * qw
                                col0 = d * 128 if d >= 0 else 0
                                nc.tensor.matmul(
                                    st[:, j, col0:],
                                    kT[po:po + 64, tr, kt * 128:(kt + 1) * 128],
                                    qT[po:po + 64, tr, qw * QW + col0:(qw + 1) * QW],
                                    start=True, stop=True)
                            # batched exp over 2 k-tiles; stale PSUM cols
                            # left of the causal edge are exp'd but never read
                            nc.scalar.activation(pt[:], st[:], AF.Exp,
                                                 scale=1.0 / math.sqrt(HD))
                            for j in range(2):
                                kt = 2 * g + j
                                d = kt - 4 * qw
                                if d >= 0:
                                    col0 = d * 128
                                    nc.gpsimd.tensor_tensor(
                                        pt[:, j, col0:col0 + 128],
                                        pt[:, j, col0:col0 + 128], trim_sb[:],
                                        op=MUL)
                            for j in range(2):
                                kt = 2 * g + j
                                col0 = (kt - 4 * qw) * 128 if kt >= 4 * qw else 0
                                nc.tensor.matmul(
                                    y_ps[:, col0:],
                                    vaug[:, kt, h, :],
                                    pt[:, j, col0:],
                                    start=(kt == 0), stop=(kt == n_kt - 1))

                        # epilogue: divide by denominator (row 64 of y_ps);
                        # broadcast 1/denom across partitions via K=1 matmul
                        rd = epil.tile([1, QW], bf16, tag="rd")
                        with nc.allow_low_precision(reason="bf16 matmul input"):
                            nc.vector.reciprocal(rd[:], y_ps[64:65, :])
                        bc_ps = bcps.tile([64, QW], f32)
                        nc.tensor.matmul(bc_ps[:], ones_sb[:], rd[:],
                                         start=True, stop=True)
                        bc = epil.tile([64, QW], bf16, tag="bc")
                        nc.vector.tensor_copy(bc[:], bc_ps[:])
                        nc.vector.tensor_tensor(
                            ybuf[po:po + 64, tr, qw * QW:(qw + 1) * QW],
                            y_ps[0:64, :], bc[:], op=MUL)

            # ---- phase 3: partial c_proj (out^T = wp^T @ ybuf) ----
            with ExitStack() as p3:
                wppool = p3.enter_context(tc.tile_pool(name="wp", bufs=1))
                wp_sb = wppool.tile([128, CG // 128, C], bf16)
                nc.sync.dma_start(wp_sb[:], wp.rearrange("(kc p) n -> p kc n", p=128))
                ops = p3.enter_context(tc.tile_pool(name="ops", bufs=3, space="PSUM"))
                ostg = p3.enter_context(tc.tile_pool(name="ostg", bufs=3))
                for mo in range(C // 128):
                    for qw in range(NQW):
                        po_ps = ops.tile([128, QW], f32)
                        for kc in range(CG // 128):
                            nc.tensor.matmul(
                                po_ps[:],
                                wp_sb[:, kc, mo * 128:(mo + 1) * 128],
                                ybuf[:, kc, qw * QW:(qw + 1) * QW],
                                start=(kc == 0), stop=(kc == CG // 128 - 1))
                        ot = ostg.tile([128, QW], f32)
                        if (mo + qw) % 2 == 0:
                            nc.scalar.copy(ot[:], po_ps[:])
                        else:
                            nc.vector.tensor_copy(ot[:], po_ps[:])
                        nc.sync.dma_start(
                            out[mo * 128:(mo + 1) * 128, qw * QW:(qw + 1) * QW],
                            ot[:])

    _split_excess_waits(nc, mybir)
    return nc


_NC_CACHE = {}


def _get_nc():
    if "nc" not in _NC_CACHE:
        _NC_CACHE["nc"] = _build_nc()
    return _NC_CACHE["nc"]


def _host_inputs(x, w_attn, w_proj):
    import ml_dtypes
    bf = ml_dtypes.bfloat16
    inv_freq = 1.0 / (10000.0 ** (np.arange(0, HD, 2, dtype=np.float32) / HD))
    t = np.arange(T, dtype=np.float32)
    freqs = np.outer(t, inv_freq)
    cos = np.cos(freqs).astype(bf)
    sin = np.sin(freqs).astype(bf)
    cosn = np.ascontiguousarray(cos.reshape(NMT, 128, HD // 2).transpose(1, 0, 2))
    sinn = np.ascontiguousarray(sin.reshape(NMT, 128, HD // 2).transpose(1, 0, 2))
    trim = np.triu(np.ones((128, 128), dtype=np.float32)).astype(bf)

    in_maps = []
    for b in range(B):
        xT = np.ascontiguousarray(x[b].T).astype(bf)  # (C, T)
        xt = np.ascontiguousarray(
            xT.reshape(NKC, 128, NMT, 128).transpose(0, 2, 1, 3))
        for hg in range(HG):
            qr = slice(hg * CG, (hg + 1) * CG)
            kr = slice(C + hg * CG, C + (hg + 1) * CG)
            vr = slice(2 * C + hg * CG, 2 * C + (hg + 1) * CG)
            wqk = np.ascontiguousarray(
                np.concatenate([w_attn[qr], w_attn[kr]], axis=0).T).astype(bf)
            wv = np.ascontiguousarray(w_attn[vr].T).astype(bf)
            wp = np.ascontiguousarray(w_proj[:, hg * CG:(hg + 1) * CG].T).astype(bf)
            in_maps.append({
                "xt": xt, "wqk": wqk, "wv": wv, "wp": wp,
                "cosn": cosn, "sinn": sinn, "trim": trim,
            })
    return in_maps


def kernel(x, w_attn, w_proj, _profile=False):
    from concourse.bass_utils import run_bass_kernel_spmd
    nc = _get_nc()
    in_maps = _host_inputs(
        np.asarray(x, dtype=np.float32),
        np.asarray(w_attn, dtype=np.float32),
        np.asarray(w_proj, dtype=np.float32))
    res = run_bass_kernel_spmd(nc, in_maps, core_ids=list(range(N_CORES)),
                               trace=_profile)
    out = np.empty((B, T, C), dtype=np.float32)
    for b in range(B):
        acc = res.results[2 * b]["o"] + res.results[2 * b + 1]["o"]
        out[b] = acc.T
    if _profile:
        return out, res
    return out


# revision 18
# speedup vs baseline: 1.2651x; 1.0060x over previous
"""Trainium2 Bass kernel for JointSelfAttention (B=4,T=2048,C=1024,H=16).

Sharding: 8 cores = 4 batches (data-parallel) x 2 head-groups of 8 heads
(tensor-parallel).  Each core computes qkv for its head group, qk-RMSNorm,
RoPE, causal attention, and a partial c_proj; the host sums the two partial
projections per batch and transposes back.

v2: bf16 compute (matmul 1.0 cyc/row, DVE 2x modes, half DMA), paired head
transposes ([128,(2h,64d)] -> [128,128], halving PE transpose rows), exp
batched x4 over PSUM score groups, Act engine stripped to exp+sqrt only,
elementwise work spread across DVE/Pool.  Softmax stays max-free (scores
bounded by sqrt(hd)=8), denominator folded into attn@v via a ones column.
"""

import math
import numpy as np
from contextlib import ExitStack

B, T, C, H, HD = 4, 2048, 1024, 16, 64
HG = 2              # head groups (tensor-parallel dim)
HPG = H // HG       # heads per group = 8
CG = HPG * HD       # channels per group = 512
N_CORES = B * HG
EPS = float(np.finfo(np.float32).eps)
QW = 512            # query window (free dim per attention block)
NQW = T // QW       # 4 windows
NKT = T // 128      # 16 k tiles
NMT = T // 128      # 16 m (token) tiles
NKC = C // 128      # 8 contraction tiles for qkv


def _split_excess_waits(nc, mybir, max_waits=1):
    """This container's walrus only encodes 1 sync-wait per instruction
    ("Too many sync wait commands" in CoreV3 codegen).  Move extra waits to
    preceding NoOps on the same engine."""
    for f in nc.m.functions:
        for bb in f.blocks:
            new_insts = []
            for inst in bb.instructions:
                si = inst.sync_info
                if si is not None and si.on_wait and len(si.on_wait) > max_waits:
                    waits = list(si.on_wait)
                    extra, keep = waits[:-max_waits], waits[-max_waits:]
                    for i in range(0, len(extra), max_waits):
                        nop = mybir.InstNoOp(
                            name=f"{inst.name}-ws{i}", ins=[], outs=[])
                        nop.engine = inst.engine
                        nop.sync_info = mybir.SyncInfo(
                            on_wait=extra[i:i + max_waits], on_update=[])
                        new_insts.append(nop)
                    inst.sync_info = mybir.SyncInfo(
                        on_wait=keep, on_update=list(si.on_update or []))
                new_insts.append(inst)
            bb.instructions.clear()
            bb.instructions.extend(new_insts)


def _build_nc():
    import concourse.bass as bass
    import concourse.tile as tile
    from concourse import mybir
    from concourse.masks import make_identity

    f32 = mybir.dt.float32
    bf16 = mybir.dt.bfloat16
    f8 = mybir.dt.float8e4
    DR = mybir.MatmulPerfMode.DoubleRow
    AF = mybir.ActivationFunctionType
    MUL = mybir.AluOpType.mult

    nc = bass.Bass("TRN2", debug=False, num_devices=N_CORES)

    xt = nc.dram_tensor("xt", [NMT, 128, NKC * 128], bf16, kind="ExternalInput").ap()
    wqk = nc.dram_tensor("wqk", [C, 2 * CG], bf16, kind="ExternalInput").ap()
    wv = nc.dram_tensor("wv", [C, CG], bf16, kind="ExternalInput").ap()
    wp = nc.dram_tensor("wp", [CG, C], bf16, kind="ExternalInput").ap()
    cosn = nc.dram_tensor("cosn", [128, NMT, HD // 2], bf16, kind="ExternalInput").ap()
    sinn = nc.dram_tensor("sinn", [128, NMT, HD // 2], bf16, kind="ExternalInput").ap()
    trim = nc.dram_tensor("trim", [128, 128], bf16, kind="ExternalInput").ap()
    out = nc.dram_tensor("o", [C, T], f32, kind="ExternalOutput").ap()

    with tile.TileContext(nc) as tc:
        with ExitStack() as ctx:
            # ---- persistent buffers (live across all phases) ----
            persist = ctx.enter_context(tc.tile_pool(name="persist", bufs=1))
            # q/k heads transposed: head h lives at partitions (h%2)*64..+64,
            # free slot h//2 -> [128, 4, T]
            qT = persist.tile([128, HPG // 2, T], bf16)
            kT = persist.tile([128, HPG // 2, T], bf16)
            vaug = persist.tile([128, NKT, HPG, HD + 1], bf16)  # v + ones col
            cos_sb = persist.tile([128, NMT, HD // 2], bf16)
            sin_sb = persist.tile([128, NMT, HD // 2], bf16)
            trim_sb = persist.tile([128, 128], bf16)
            ident = persist.tile([128, 128], bf16)
            eps_sb = persist.tile([128, 1], f32)

            nc.sync.dma_start(cos_sb[:], cosn[:])
            nc.sync.dma_start(sin_sb[:], sinn[:])
            nc.sync.dma_start(trim_sb[:], trim[:])
            make_identity(nc, ident[:])
            nc.vector.memset(eps_sb[:], EPS)
            nc.gpsimd.memset(
                vaug[:, :, :, HD:HD + 1].rearrange("p a b one -> p (a b one)"), 1.0)

            # ---- phase 1: qkv + norm + rope + transpose ----
            with ExitStack() as p1:
                wpool = p1.enter_context(tc.tile_pool(name="w1", bufs=1))
                wqk_sb = wpool.tile([128, NKC, 2 * CG], bf16)
                wv_sb = wpool.tile([128, NKC, CG], bf16)
                nc.sync.dma_start(wqk_sb[:], wqk.rearrange("(kc p) n -> p kc n", p=128))
                nc.sync.dma_start(wv_sb[:], wv.rearrange("(kc p) n -> p kc n", p=128))

                xpool = p1.enter_context(tc.tile_pool(name="xp", bufs=3))
                qkps = p1.enter_context(tc.tile_pool(name="qkps", bufs=2, space="PSUM"))
                vps = p1.enter_context(tc.tile_pool(name="vps", bufs=2, space="PSUM"))
                tpps = p1.enter_context(tc.tile_pool(name="tpps", bufs=2, space="PSUM"))
                work = p1.enter_context(tc.tile_pool(name="work", bufs=2))
                stats = p1.enter_context(tc.tile_pool(name="stats", bufs=2))

                for mt in range(NMT):
                    xt_sb = xpool.tile([128, NKC, 128], bf16, tag="xt")
                    nc.sync.dma_start(
                        xt_sb[:].rearrange("p kc t -> p (kc t)"), xt[mt])

                    qk_ps = qkps.tile([128, 2 * CG], f32)
                    for n in range(2):
                        for kc in range(NKC):
                            nc.tensor.matmul(
                                qk_ps[:, n * CG:(n + 1) * CG],
                                xt_sb[:, kc, :],
                                wqk_sb[:, kc, n * CG:(n + 1) * CG],
                                start=(kc == 0), stop=(kc == NKC - 1))
                    v_ps = vps.tile([128, CG], f32)
                    for kc in range(NKC):
                        nc.tensor.matmul(
                            v_ps[:], xt_sb[:, kc, :], wv_sb[:, kc, :],
                            start=(kc == 0), stop=(kc == NKC - 1))

                    # v -> vaug (strided per-head copy, leaves ones col intact)
                    nc.scalar.copy(
                        vaug[:, mt, :, 0:HD],
                        v_ps[:].rearrange("p (h d) -> p h d", d=HD))

                    # qk RMS norm: rr = 1/sqrt(mean(x^2)+eps) per (token, head)
                    sq = work.tile([128, 2 * CG], bf16, tag="sq")
                    nc.scalar.activation(sq[:], qk_ps[:], AF.Square)
                    ss = stats.tile([128, 2 * HPG], f32)
                    nc.vector.tensor_reduce(
                        ss[:], sq[:].rearrange("p (h d) -> p h d", d=HD),
                        axis=mybir.AxisListType.X, op=mybir.AluOpType.add)
                    rr = stats.tile([128, 2 * HPG], f32)
                    nc.scalar.activation(rr[:], ss[:], AF.Sqrt,
                                         bias=eps_sb[:], scale=1.0 / HD)
                    nc.vector.reciprocal(rr[:], rr[:])
                    # norm multiply doubles as PSUM->SBUF bf16 conversion
                    qkn = work.tile([128, 2 * CG], bf16, tag="qkn")
                    rr_b = bass.AP(tensor=rr.tensor, offset=rr.offset,
                                   ap=[rr.ap[0], [1, 2 * HPG], [0, HD]])
                    nc.vector.tensor_tensor(
                        qkn[:].rearrange("p (h d) -> p h d", d=HD),
                        qk_ps[:].rearrange("p (h d) -> p h d", d=HD),
                        rr_b, op=MUL)

                    # rope on q and k at once: view [128, (2sec 8h 2half 32)]
                    # y1 = x1 c + x2 s ; y2 = x2 c - x1 s  (write back in place;
                    # the adds/subs only read the tmps)
                    base = qkn[:].rearrange(
                        "p (s h two d) -> p s h two d", s=2, h=HPG, two=2)
                    x1, x2 = base[:, :, :, 0, :], base[:, :, :, 1, :]
                    cb = bass.AP(tensor=cos_sb.tensor,
                                 offset=cos_sb.offset + mt * (HD // 2),
                                 ap=[cos_sb.ap[0], [0, 2], [0, HPG], [1, HD // 2]])
                    sb_ = bass.AP(tensor=sin_sb.tensor,
                                  offset=sin_sb.offset + mt * (HD // 2),
                                  ap=[sin_sb.ap[0], [0, 2], [0, HPG], [1, HD // 2]])
                    t1 = work.tile([128, 2, HPG, HD // 2], bf16, tag="rt1")
                    t2 = work.tile([128, 2, HPG, HD // 2], bf16, tag="rt2")
                    t3 = work.tile([128, 2, HPG, HD // 2], bf16, tag="rt3")
                    t4 = work.tile([128, 2, HPG, HD // 2], bf16, tag="rt4")
                    nc.vector.tensor_tensor(t1[:], x1, cb, op=MUL)
                    nc.vector.tensor_tensor(t2[:], x2, sb_, op=MUL)
                    nc.vector.tensor_tensor(t3[:], x2, cb, op=MUL)
                    nc.vector.tensor_tensor(t4[:], x1, sb_, op=MUL)
                    nc.vector.tensor_add(x1, t1[:], t2[:])
                    nc.vector.tensor_sub(x2, t3[:], t4[:])

                    # transpose head pairs: [128 tok, (2h, 64d)] -> [128, 128]
                    # (partitions 0-63 = head 2j dims, 64-127 = head 2j+1)
                    for sec, dstT in ((0, qT), (1, kT)):
                        tp = tpps.tile([128, HPG // 2, 128], bf16, tag="tp")
                        for j in range(HPG // 2):
                            src = qkn[:, sec * CG + 2 * j * HD:
                                      sec * CG + (2 * j + 2) * HD]
                            nc.tensor.transpose(
                                tp[:, j, :],
                                src.rearrange("p (two d) -> p two d", two=2),
                                ident[:])
                        nc.vector.tensor_copy(
                            dstT[:, :, mt * 128:(mt + 1) * 128], tp[:])

            # ---- phases 2+3 share ybuf (allocated after phase-1 weights free) ----
            ybufp = ctx.enter_context(tc.tile_pool(name="ybufp", bufs=1))
            ybuf = ybufp.tile([128, CG // 128, T], bf16)  # attn out (ch x T)

            # ---- phase 2: attention per (head, q-window) ----
            # scores/exp stay in (k x q) layout; attn@v runs transposed
            # (stationary = 128x128 pt block, moving = vaug [128,65]) so each
            # block costs only 65 PE rows and the softmax denominator lands
            # per-partition, making the division a cheap free-dim broadcast.
            with ExitStack() as p2:
                stps = p2.enter_context(tc.tile_pool(name="stps", bufs=2, space="PSUM"))
                ytps = p2.enter_context(tc.tile_pool(name="ytps", bufs=2, space="PSUM"))
                ytpps = p2.enter_context(tc.tile_pool(name="ytpps", bufs=2, space="PSUM"))
                ptpool = p2.enter_context(tc.tile_pool(name="pt", bufs=2))
                epil = p2.enter_context(tc.tile_pool(name="epil", bufs=2))

                for h in range(HPG):
                    po = (h % 2) * 64
                    tr = h // 2
                    for qw in range(NQW):
                        n_kt = 4 * qw + 4
                        pt = ptpool.tile([128, NKT, QW], bf16, tag="pt")
                        for g in range(n_kt // 2):
                            st = stps.tile([128, 2, QW], f32, tag="st")
                            for j in range(2):
                                kt = 2 * g + j
                                d = kt - 4 * qw
                                col0 = d * 128 if d >= 0 else 0
                                nc.tensor.matmul(
                                    st[:, j, col0:],
                                    kT[po:po + 64, tr, kt * 128:(kt + 1) * 128],
                                    qT[po:po + 64, tr, qw * QW + col0:(qw + 1) * QW],
                                    start=True, stop=True)
                            # batched exp over 2 k-tiles; stale PSUM cols
                            # left of the causal edge are exp'd but never read.
                            # The 2nd diagonal pair only has cols 256+ live.
                            ec = 256 if 2 * g - 4 * qw >= 2 else 0
                            nc.scalar.activation(
                                pt[:, 2 * g:2 * g + 2, ec:], st[:, :, ec:],
                                AF.Exp, scale=1.0 / math.sqrt(HD))
                            for j in range(2):
                                kt = 2 * g + j
                                d = kt - 4 * qw
                                if d >= 0:
                                    col0 = d * 128
                                    nc.gpsimd.tensor_tensor(
                                        pt[:, kt, col0:col0 + 128],
                                        pt[:, kt, col0:col0 + 128], trim_sb[:],
                                        op=MUL)

                        # transposed attn@v: one [128q, 65] chain per q-tile
                        yT = ytps.tile([128, 4, HD + 1], f32)
                        for i in range(4):
                            qt = 4 * qw + i
                            for kt in range(qt + 1):
                                nc.tensor.matmul(
                                    yT[:, i, :],
                                    pt[:, kt, i * 128:(i + 1) * 128],
                                    vaug[:, kt, h, :],
                                    start=(kt == 0), stop=(kt == qt))

                        # divide by denominator (col 64, per-partition)
                        rd = epil.tile([128, 4], f32, tag="rd")
                        nc.vector.reciprocal(rd[:], yT[:, :, HD])
                        rd_b = bass.AP(tensor=rd.tensor, offset=rd.offset,
                                       ap=[rd.ap[0], [1, 4], [0, HD]])
                        ysb = epil.tile([128, 4, HD], bf16, tag="ysb")
                        nc.vector.tensor_tensor(ysb[:], yT[:, :, 0:HD], rd_b,
                                                op=MUL)
                        # back to (ch x T) for the projection
                        ytp = ytpps.tile([64, 4, 128], bf16)
                        for i in range(4):
                            nc.tensor.transpose(ytp[:, i, :], ysb[:, i, :],
                                                ident[:])
                        nc.vector.tensor_copy(
                            ybuf[po:po + 64, tr, qw * QW:(qw + 1) * QW]
                            .rearrange("p (i t) -> p i t", t=128),
                            ytp[:])

            # ---- phase 3: partial c_proj (out^T = wp^T @ ybuf) ----
            with ExitStack() as p3:
                wppool = p3.enter_context(tc.tile_pool(name="wp", bufs=1))
                wp_sb = wppool.tile([128, CG // 128, C], bf16)
                nc.sync.dma_start(wp_sb[:], wp.rearrange("(kc p) n -> p kc n", p=128))
                ops = p3.enter_context(tc.tile_pool(name="ops", bufs=3, space="PSUM"))
                ostg = p3.enter_context(tc.tile_pool(name="ostg", bufs=3))
                for mo in range(C // 128):
                    for qw in range(NQW):
                        po_ps = ops.tile([128, QW], f32)
                        for kc in range(CG // 128):
                            nc.tensor.matmul(
                                po_ps[:],
                                wp_sb[:, kc, mo * 128:(mo + 1) * 128],
                                ybuf[:, kc, qw * QW:(qw + 1) * QW],
                                start=(kc == 0), stop=(kc == CG // 128 - 1))
                        ot = ostg.tile([128, QW], f32)
                        if (mo + qw) % 2 == 0:
                            nc.scalar.copy(ot[:], po_ps[:])
                        else:
                            nc.vector.tensor_copy(ot[:], po_ps[:])
                        nc.sync.dma_start(
                            out[mo * 128:(mo + 1) * 128, qw * QW:(qw + 1) * QW],
                            ot[:])

    _split_excess_waits(nc, mybir)
    return nc


_NC_CACHE = {}


def _get_nc():
    if "nc" not in _NC_CACHE:
        _NC_CACHE["nc"] = _build_nc()
    return _NC_CACHE["nc"]


def _host_inputs(x, w_attn, w_proj):
    import ml_dtypes
    bf = ml_dtypes.bfloat16
    inv_freq = 1.0 / (10000.0 ** (np.arange(0, HD, 2, dtype=np.float32) / HD))
    t = np.arange(T, dtype=np.float32)
    freqs = np.outer(t, inv_freq)
    cos = np.cos(freqs).astype(bf)
    sin = np.sin(freqs).astype(bf)
    cosn = np.ascontiguousarray(cos.reshape(NMT, 128, HD // 2).transpose(1, 0, 2))
    sinn = np.ascontiguousarray(sin.reshape(NMT, 128, HD // 2).transpose(1, 0, 2))
    trim = np.triu(np.ones((128, 128), dtype=np.float32)).astype(bf)

    in_maps = []
    for b in range(B):
        xT = np.ascontiguousarray(x[b].T)  # (C, T)
        # [mt, ch-in-chunk, kc*128+tok]: one contiguous DMA per token tile,
        # partitions carry the contraction channels
        xt = np.ascontiguousarray(
            xT.reshape(NKC, 128, NMT, 128).transpose(2, 1, 0, 3)
        ).reshape(NMT, 128, NKC * 128).astype(bf)
        for hg in range(HG):
            qr = slice(hg * CG, (hg + 1) * CG)
            kr = slice(C + hg * CG, C + (hg + 1) * CG)
            vr = slice(2 * C + hg * CG, 2 * C + (hg + 1) * CG)
            wqk = np.ascontiguousarray(
                np.concatenate([w_attn[qr], w_attn[kr]], axis=0).T).astype(bf)
            wv = np.ascontiguousarray(w_attn[vr].T).astype(bf)
            wp = np.ascontiguousarray(w_proj[:, hg * CG:(hg + 1) * CG].T).astype(bf)
            in_maps.append({
                "xt": xt, "wqk": wqk, "wv": wv, "wp": wp,
                "cosn": cosn, "sinn": sinn, "trim": trim,
            })
    return in_maps


def kernel(x, w_attn, w_proj, _profile=False):
    from concourse.bass_utils import run_bass_kernel_spmd
    nc = _get_nc()
    in_maps = _host_inputs(
        np.asarray(x, dtype=np.float32),
        np.asarray(w_attn, dtype=np.float32),
        np.asarray(w_proj, dtype=np.float32))
    res = run_bass_kernel_spmd(nc, in_maps, core_ids=list(range(N_CORES)),
                               trace=_profile)
    out = np.empty((B, T, C), dtype=np.float32)
    for b in range(B):
        acc = res.results[2 * b]["o"] + res.results[2 * b + 1]["o"]
        out[b] = acc.T
    if _profile:
        return out, res
    return out
